# revision 67
# baseline (speedup 1.0000x reference)
"""AttnBlock (VAE-style single-head spatial attention) on 8 Trainium2 cores.

Problem: x[B=4, C=512, H=64, W=64]; qkv 1x1-conv -> attention over N=H*W=4096
tokens -> proj 1x1-conv -> residual add.

Sharding: 8 cores = 4 batch images x 2 query-halves. Each core handles the
full 4096-token context (K/V) of one image and 2048 of its queries. Per-core
x columns are rotated so the query half is always columns [0, 2048) -- the
kj context order is irrelevant (summed over), so the SPMD program is
identical on every core.

Host-side folding (all cheap 512x512 ops):
 - K-bias adds a per-query constant to every logit -> cancels in softmax.
 - V-bias contributes exactly bv to every output column (softmax rows sum to
   1) -> folded with proj_b into the residual tensor xresb = x_q + beff.
 - S^T[kj,qi] = x^T (Wk^T (Wq x_q + bq)) = x^T (W2 x_q + b2) with
   W2 = Wk^T Wq, b2 = Wk^T bq. Scores are computed TRANSPOSED directly from
   x -- no K tensor and no on-chip transposes.
 - Logits are tiny here (|s| < ~1.5), so softmax needs no max-subtraction.

 - The V projection is folded INTO the output projection: out =
   proj(Wv X a) = W3 (X a) with W3 = proj_w @ wv. The kernel therefore
   never materializes V at all -- it accumulates G = X @ p directly from a
   host-provided x^T copy, eliminating all V^T matmuls.

Precision plan: all heavy matmuls run in fp8(e4m3) with DoubleRow perf
mode -- the PE contracts 256 channels per instruction at the same
instruction cost as a 128-deep bf16 matmul, i.e. 2x throughput. w2T/w3T
are scaled by 64 and x^T by 32 on the host so everything lands in e4m3's
normal range; the scales are removed via the exp() scale argument and a
fused 1/2048 multiply in the output op. PSUM accumulation stays fp32 and
the residual add uses an exact fp32 x + beff tensor, so the output error
stays ~2e-4 relative (residual-dominated output).

Per query tile (512 queries), context loop of 16 chunk-PAIRS (2x128
tokens): S^T (2 DoubleRow matmuls/chunk, fp32 PSUM) -> exp on ACT (fp8
out, written into pair-tiles) -> G accumulate (4 DoubleRow matmuls/pair,
lhsT = x^T chunks) + a ones-row DoubleRow matmul accumulating the softmax
denominator in PSUM. In the epilogue the softmax 1/denominator (fast
~51-ULP reciprocal + GPSIMD partition broadcast) is folded into the
G->fp8 normalize mul BEFORE the W3 matmul, so the output-projection PSUM
is exactly 2048x the true attention output and the output stage is a
single fused (pr/2048 + xres) DVE op per co-tile followed by its store.

Pipelining: all four Q' tiles are computed upfront (they only need w2T +
x half 0, covering the bulk-DMA window at the head behind a short PE
warmup spin that also lifts the HAM clock throttle); tile 0's G matmuls
trail its S^T by 4 pairs so the first x^T chunks can land; each tile's
epilogue is covered by the next tile's S^T/exp work with its G matmuls
deferred past the W3 groups; and the LAST tile processes context pairs
0,1 at the very end (their exps finished ~25us earlier) so the final den
stop -> recip -> broadcast -> normalize chain rides exp-independent PE
work. Head input DMA: small critical prefix split across both HWDGE
queues, then junk loads burn the ~10 rotating flow-control semaphores so
the bulk transfers cannot steal DMA engines from the prefix; bulk issue
stays off the scalar queue so Q' bias ACTIVATEs are never stuck behind
0.6-8us DMA descriptor-generation instructions.
"""

import os

import numpy as np

B, C = 4, 512
N = 4096          # H*W tokens
QH = N // 2       # queries per core
QT = 512          # query tile (free dim of most matmuls)
NQT = QH // QT    # 4 query tiles per core
NKC = N // 128    # 32 context chunks
NPR = NKC // 2    # 16 context chunk-pairs
NCC = C // 128    # 4 channel chunks
NCORES = 8
OVERLAP = 4       # next-tile chunk-pairs deferred into the epilogue window
                  # (must equal NCC: one deferred PV group per proj group)
WSCALE = 64.0     # host weight scale into fp8 range

_COMPILED = None
LAST_RESULTS = None  # stashed BassKernelResults for test harness inspection


def _build():
    import concourse.bass as bass  # noqa: F401
    import concourse.mybir as mybir
    import concourse.tile as tile
    from concourse import bacc

    f32 = mybir.dt.float32
    f16 = mybir.dt.float16
    fp8 = mybir.dt.float8e4
    bf16 = mybir.dt.bfloat16
    EXP = mybir.ActivationFunctionType.Exp
    MUL = mybir.AluOpType.mult
    ADD = mybir.AluOpType.add
    DR = mybir.MatmulPerfMode.DoubleRow
    escale = float(C) ** -0.5 / WSCALE
    # G = X @ p is normalized BEFORE the folded output matmul: gmat =
    # G * (1/den) lands at ~0.6-sigma in fp8 (xT carries a host-side x32),
    # and W3 = proj_w @ wv is folded on the host (x64 into fp8), so the
    # PSUM output is exactly 2048x the true attention output. The residual
    # stage folds the 1/2048 into a single fused (pr*s + xres) DVE op.
    OUTSCALE = 1.0 / (WSCALE * 32.0)

    nc = bacc.Bacc("TRN2", target_bir_lowering=False, debug=False,
                   num_devices=NCORES)

    # DRAM I/O (per-core shapes)
    xin = nc.dram_tensor("xin", [C, N], fp8, kind="ExternalInput")
    xT = nc.dram_tensor("xT", [N, C], fp8, kind="ExternalInput")
    xresb = nc.dram_tensor("xresb", [C, QH], f32, kind="ExternalInput")
    w2T = nc.dram_tensor("w2T", [C, C], fp8, kind="ExternalInput")
    w3T = nc.dram_tensor("w3T", [C, C], fp8, kind="ExternalInput")
    b2 = nc.dram_tensor("b2", [C], f32, kind="ExternalInput")
    y = nc.dram_tensor("y", [C, QH], f32, kind="ExternalOutput")

    xr = xin.ap().rearrange("(t p) n -> p t n", p=128)      # [128, 4, 4096]
    xqr = xresb.ap().rearrange("(t p) n -> p t n", p=128)   # [128, 4, 2048]
    yr = y.ap().rearrange("(t p) n -> p t n", p=128)        # [128, 4, 2048]

    with tile.TileContext(nc) as tc:
        with (
            tc.tile_pool(name="singles", bufs=1) as singles,
            tc.tile_pool(name="qp", bufs=4) as qp_pool,
            tc.tile_pool(name="pt", bufs=6) as pt_pool,
            tc.tile_pool(name="hms", bufs=2) as hms_pool,
            tc.tile_pool(name="xres", bufs=2) as xres_pool,
            tc.tile_pool(name="outp", bufs=2) as out_pool,
            tc.tile_pool(name="rc", bufs=2) as rc_pool,
            tc.tile_pool(name="dacc", bufs=2) as dacc_pool,
            tc.tile_pool(name="work", bufs=3, space="PSUM") as work_pool,
            tc.tile_pool(name="hm", bufs=1, space="PSUM") as hm_pool,
            tc.tile_pool(name="den", bufs=1, space="PSUM") as den_pool,
        ):
            # --- PE warmup: ~4.5us of dependency-free matmuls ----------
            # The HAM clock gate needs ~3.4us of sustained PE activity to
            # lift the 1.2 GHz cold throttle; these run during the input
            # DMA wait so the real matmuls start at 2.4 GHz.
            wu_sb = singles.tile([128, QT], bf16)
            nc.vector.memset(wu_sb, 0.0)
            ones_bf = singles.tile([128, 1], bf16)
            nc.vector.memset(ones_bf, 1.0)
            ones16 = singles.tile([128, 1], f16)
            nc.vector.memset(ones16, 1.0)
            wu_keep = singles.tile([1, QT], f32)
            NWU = 13  # bridges engine boot + first-input DMA wait (the
            # head loads land ~11.5us in with dual-queue issue) while
            # giving the PE the ~3us of sustained work its p-state ramp
            # needs
            for w in range(NWU):
                wu_ps = work_pool.tile([1, QT], f32, tag="work", name="wu_ps")
                nc.tensor.matmul(wu_ps, lhsT=ones_bf, rhs=wu_sb)
                if w == NWU - 1:  # keep the chain live against DCE
                    nc.vector.tensor_copy(wu_keep, wu_ps)

            # --- DMAs in consumption-priority order ---------------------
            # The input fetch is descriptor-rate-bound (~3.6ns per 512B x
            # row => ~142GB/s on one queue), so the head loads alternate
            # between BOTH HWDGE queues (sync + scalar; ACT is idle until
            # the first exps) for ~2x aggregate fetch rate.
            _dmaq = [nc.sync, nc.scalar]
            _dqi = [0]

            def head_dma(out, in_):
                _dmaq[_dqi[0] % 2].dma_start(out=out, in_=in_)
                _dqi[0] += 1

            # two pair-tiles: Q'(0)'s first matmul starts after a 128KB DMA
            # instead of the full 256KB weight load
            w2Tr = w2T.ap().rearrange("(t p) m -> p t m", p=128)
            w2T_sb = []
            for tp in range(2):
                ww = singles.tile([128, 2, C], fp8, name=f"w2T{tp}")
                head_dma(out=ww, in_=w2Tr[:, 2 * tp:2 * tp + 2, :])
                w2T_sb.append(ww)
            x00 = []
            x01 = []
            for tp in range(2):
                xx = singles.tile([128, 2, QT], fp8, name=f"x00{tp}")
                head_dma(out=xx, in_=xr[:, 2 * tp:2 * tp + 2, 0:QT])
                x00.append(xx)
            for tp in range(2):
                xx = singles.tile([128, 2, QT], fp8, name=f"x01{tp}")
                head_dma(out=xx, in_=xr[:, 2 * tp:2 * tp + 2, QT:2 * QT])
                x01.append(xx)
            b2_sb = singles.tile([128, NCC], f32)
            nc.sync.dma_start(out=b2_sb,
                              in_=b2.ap().rearrange("(t p) -> p t", p=128))

            # x (fp8): [half][512-col group g] -> [128, 2(ci-pair), 512]
            # per tp. Group 0 of half 0 is its own small pair of tiles so
            # Q'(0) starts after 2x128KB; the rest load as two big tiles
            # per half (1536/2048-col) -- DMA issue instructions cost
            # ~0.6us each regardless of size, so fewer+bigger transfers
            # drain the head ~3x faster.
            xg = [[None] * 4 for _ in range(2)]
            xg[0][0] = x00
            xg[0][1] = x01
            # Everything below the prefix issues on SYNC ONLY: a DMA
            # issue instruction costs 0.6-8us of engine time (it is
            # descriptor-row-bound), and issues on the scalar queue would
            # delay the Q' bias-add ACTIVATEs behind them, stalling the
            # PE at the head.
            # The ~10 rotating DMA flow-control semaphores make every DMA
            # instruction wait for the completion of the one issued ~10
            # earlier. These tiny junk loads burn the remaining slots so
            # the BULK loads below cannot start transferring (and
            # stealing shared DMA engines) until the startup-critical
            # prefix above has fully landed.
            junk_sb = singles.tile([128, 7], f32, name="junk")
            for jx in range(7):
                c = jx % 4
                nc.sync.dma_start(out=junk_sb[:, jx:jx + 1],
                                  in_=b2.ap().rearrange("(t p) -> p t",
                                                        p=128)[:, c:c + 1])
            # Bulk loads, ordered by first consumption: x half-0 groups 2-3
            # (S^T pair 4 at ~15.5us), x^T chunks 0-7 (tile-0 PV pairs 0-3
            # trail by 4 pairs; x32 host scale), x half-1 (S^T pair 8),
            # then the remaining x^T chunks (PV pair 4 onward).
            xTr = xT.ap().rearrange("(j p) c -> p j c", p=128)
            xT_sb = singles.tile([128, NKC, C], fp8, name="xT")
            xbig = [[None, None], [None, None]]  # [h][tp]
            for tp in range(2):
                xb = singles.tile([128, 2, 2 * QT], fp8, name=f"xb0{tp}")
                nc.sync.dma_start(
                    out=xb, in_=xr[:, 2 * tp:2 * tp + 2, 2 * QT:4 * QT])
                xbig[0][tp] = xb
            nc.sync.dma_start(out=xT_sb[:, 0:4, :], in_=xTr[:, 0:4, :])
            nc.sync.dma_start(out=xT_sb[:, 4:8, :], in_=xTr[:, 4:8, :])
            for tp in range(2):
                xb = singles.tile([128, 2, 4 * QT], fp8, name=f"xb1{tp}")
                nc.sync.dma_start(
                    out=xb, in_=xr[:, 2 * tp:2 * tp + 2, QH:QH + 4 * QT])
                xbig[1][tp] = xb
            nc.sync.dma_start(out=xT_sb[:, 8:16, :], in_=xTr[:, 8:16, :])
            nc.sync.dma_start(out=xT_sb[:, 16:24, :], in_=xTr[:, 16:24, :])
            nc.sync.dma_start(out=xT_sb[:, 24:NKC, :],
                              in_=xTr[:, 24:NKC, :])

            def xga(h, g, tp, a=0, b=QT):
                # AP for columns [a,b) of 512-col group g of half h
                if h == 0 and g < 2:
                    return xg[0][g][tp][:, :, a:b]
                off = ((g - 2) if h == 0 else g) * QT
                return xbig[h][tp][:, :, off + a:off + b]
            w3T_sb = singles.tile([128, NCC, C], fp8)
            nc.sync.dma_start(
                out=w3T_sb,
                in_=w3T.ap().rearrange("(t p) m -> p t m", p=128))
            # fp8 ones for the DoubleRow denominator matmul; width 16 keeps
            # the pair-dim step a multiple of 16 as the DR AP rules require
            ones8 = singles.tile([128, 2, 16], fp8)
            nc.vector.memset(ones8, 1.0)

            def xchunk(j):  # lhsT [ci, 2, kj-cols] fp8 for context chunk j
                h, r = divmod(j, 16)
                g, o = divmod(r, 4)
                return (lambda tp: xga(h, g, tp,
                                       o * 128, (o + 1) * 128))

            S = {}  # per-q live tiles

            def emit_A(q):  # Q' = W2 @ x_q + b2 (scaled by 64, fp8 out)
                # two pair-tiles (not one) so the first S^T matmul of this
                # tile waits only on its own pair's bias adds
                qp_sb = [
                    qp_pool.tile([128, 2, QT], fp8, tag=f"qp{h}",
                                 name=f"qp{q}_{h}")
                    for h in range(2)
                ]
                for m in range(NCC):
                    qp_ps = work_pool.tile([128, QT], f32, tag="work",
                                           name="qp_ps")
                    for tp in range(2):
                        nc.tensor.matmul(
                            qp_ps,
                            lhsT=w2T_sb[tp][:, :, m * 128:(m + 1) * 128],
                            rhs=xga(0, q, tp),
                            start=(tp == 0), stop=(tp == 1),
                            perf_mode=DR,
                        )
                    # bias-add on ACT (Identity), which is idle exactly at
                    # tile boundaries -- the DVE queue there is busy with
                    # the epilogue and would delay the first S^T matmuls
                    nc.scalar.add(
                        qp_sb[m // 2][:, m % 2, :], qp_ps,
                        b2_sb[:, m:m + 1])
                S[q] = {"qp": qp_sb}

            def emit_B_st(q, J):  # S^T + exp of one context chunk-pair
                if J == 0:
                    # four 1-bank tiles (not one 4-bank tile) so each
                    # normalize copy starts as soon as its own bank's last
                    # PV matmul lands, not when the whole group finishes
                    S[q]["hm"] = [
                        hm_pool.tile([128, QT], f32, tag=f"hm{m}",
                                     name=f"hm{q}_{m}")
                        for m in range(NCC)
                    ]
                    S[q]["den"] = den_pool.tile([1, QT], f32, tag="den",
                                                name=f"den{q}")
                    if q < NQT - 1:
                        S[q]["dacc"] = dacc_pool.tile(
                            [128, QT], f16, tag="dacc", name=f"dacc{q}")
                    S[q]["pt"] = {}
                if J == 4:  # prefetch fp32 residual (+bias) slice mid-loop
                    xres_sb = xres_pool.tile([128, NCC, QT], f32, tag="xres",
                                             name=f"xres{q}")
                    nc.sync.dma_start(
                        out=xres_sb, in_=xqr[:, :, q * QT:(q + 1) * QT])
                    S[q]["xres"] = xres_sb
                qp_sb = S[q]["qp"]
                ptd = pt_pool.tile([128, 2, QT], fp8, tag="pt", name="ptd")
                for jj in range(2):
                    j = 2 * J + jj
                    xs = xchunk(j)
                    st_ps = work_pool.tile([128, QT], f32, tag="work",
                                           name="st_ps")
                    for tp in range(2):
                        nc.tensor.matmul(
                            st_ps, lhsT=xs(tp),
                            rhs=qp_sb[tp],
                            start=(tp == 0), stop=(tp == 1),
                            perf_mode=DR,
                        )
                    nc.scalar.activation(ptd[:, jj, :], st_ps, EXP,
                                         scale=escale)
                S[q]["pt"][J] = ptd

            def emit_B_den(q, J, start=None, stop=None):
                # sum-exp accumulate for one chunk-pair. On tiles
                # 0..NQT-2 the EVEN pairs accumulate as DVE adds into an
                # fp16 SBUF tile (the DVE has ~4x slack vs the PE; fp16
                # keeps its 2x 16-bit path and den~4500 only needs ~0.3%
                # accuracy) and only the ODD pairs stay as PE ones-matmuls
                # -- halving the PE's den cost. A single cheap fp16
                # colsum matmul (emit_den_fold) folds the DVE accumulator
                # into the same den PSUM group at tile end. The last tile
                # keeps the all-PE path: its den feeds the latency-
                # critical final recip->broadcast->normalize chain.
                if q < NQT - 1 and J % 2 == 0:
                    ptd = S[q]["pt"][J]
                    acc = S[q]["dacc"]
                    if J == 0:
                        nc.vector.tensor_add(acc, ptd[:, 0, :],
                                             ptd[:, 1, :])
                    else:
                        nc.vector.tensor_add(acc, acc, ptd[:, 0, :])
                        nc.vector.tensor_add(acc, acc, ptd[:, 1, :])
                    return
                if q < NQT - 1:
                    start = J == 1
                    stop = False
                nc.tensor.matmul(
                    S[q]["den"],
                    lhsT=ones8[:, :, 0:1],
                    rhs=S[q]["pt"][J],
                    start=(J == 0) if start is None else start,
                    stop=(J == NPR - 1) if stop is None else stop,
                    perf_mode=DR,
                    skip_group_check=True,
                )

            def emit_den_fold(q):
                # close tile q's den PSUM group: one plain fp16 matmul
                # adds the DVE accumulator's column sums. Emitted AFTER
                # the next tile's S^T batch so the in-order PE has cover
                # while the last DVE adds drain.
                nc.tensor.matmul(S[q]["den"], lhsT=ones16,
                                 rhs=S[q]["dacc"], start=False, stop=True,
                                 skip_group_check=True)

            def emit_B_pv(q, J, den=True, start=None, stop=None):
                # PV accumulate (+ sum-exp). start/stop override the
                # J-based accumulation flags when the last tile's pairs
                # are processed out of order.
                if den:
                    emit_B_den(q, J, start=start, stop=stop)
                hm_ps = S[q]["hm"]
                ptd = S[q]["pt"].pop(J)
                for m in range(NCC):
                    nc.tensor.matmul(
                        hm_ps[m],
                        lhsT=xT_sb[:, 2 * J:2 * J + 2,
                                   m * 128:(m + 1) * 128],
                        rhs=ptd,
                        start=(J == 0) if start is None else start,
                        stop=(J == NPR - 1) if stop is None else stop,
                        perf_mode=DR,
                        skip_group_check=True,
                    )

            def emit_B(q, J, **fl):
                emit_B_st(q, J)
                emit_B_pv(q, J, **fl)

            def emit_C_head(q):
                den_ps = S[q]["den"]
                rec_sb = rc_pool.tile([1, QT], f32, tag="rec",
                                      name=f"rec{q}")
                # ~51-ULP approx (rel err ~4e-6) at 5x the Newton recip
                # speed; den ~ 4096 is far from every undefined edge case.
                # (GPSIMD cannot read PSUM, so recip must precede broadcast.)
                nc.vector.reciprocal_approx_fast(out=rec_sb, in_=den_ps)
                rbc_sb = rc_pool.tile([128, QT], f32, tag="rbc",
                                      name=f"rbc{q}")
                nc.gpsimd.partition_broadcast(rbc_sb, rec_sb)
                # two pair-tiles so each proj DoubleRow matmul waits only on
                # its own pair's normalize muls, not all four
                hmat_sb = [
                    hms_pool.tile([128, 2, QT], fp8, tag=f"hms{h}",
                                  name=f"hms{q}_{h}")
                    for h in range(2)
                ]
                for m in range(NCC):
                    dst = hmat_sb[m // 2][:, m % 2, :]
                    nc.vector.tensor_mul(dst, S[q]["hm"][m], rbc_sb)
                S[q]["hmat"] = hmat_sb

            def emit_C_tail(q, filler=None):
                # proj, then ONE fused (pr*1/4096 + xres) DVE op per
                # co-tile, then store. `filler(o)` injects independent PE
                # work (the deferred next-tile PV groups) between proj
                # groups so the in-order PE never waits on the DVE draining
                # a shared PSUM work slot.
                hmat_sb, xres_sb = S[q]["hmat"], S[q]["xres"]
                out_sb = out_pool.tile([128, NCC, QT], f32, tag="out",
                                       name=f"out{q}")
                for o in range(NCC):
                    pr_ps = work_pool.tile([128, QT], f32, tag="work",
                                           name="pr_ps")
                    for tp in range(2):
                        nc.tensor.matmul(
                            pr_ps,
                            lhsT=w3T_sb[:, 2 * tp:2 * tp + 2,
                                        o * 128:(o + 1) * 128],
                            rhs=hmat_sb[tp],
                            start=(tp == 0), stop=(tp == 1),
                            perf_mode=DR,
                        )
                    # last tile: the very last co-tile drains in two halves
                    # so the final residual op + DMA transfer are half-size
                    H = QT // 2
                    parts = [(0, QT)] if (filler is not None
                                          or o < NCC - 1) else [(0, H),
                                                                (H, QT)]
                    for a, bnd in parts:
                        nc.vector.scalar_tensor_tensor(
                            out=out_sb[:, o, a:bnd], in0=pr_ps[:, a:bnd],
                            scalar=OUTSCALE, in1=xres_sb[:, o, a:bnd],
                            op0=MUL, op1=ADD)
                        # per-co-tile store so output streams out during
                        # the remaining epilogue instead of after all of it.
                        # Last tile alternates the two HWDGE queues (ACT is
                        # idle by then) so the final flush is half as deep.
                        eng = (nc.scalar if filler is None and o % 2
                               else nc.sync)
                        eng.dma_start(
                            out=yr[:, o, q * QT + a:q * QT + bnd],
                            in_=out_sb[:, o, a:bnd])
                    if filler is not None:
                        filler(o)
                del S[q]

            # Pipeline: during tile q's epilogue (denominator -> normalize
            # -> proj), the PE stream holds only dependency-free work from
            # tile q+1 (Q' and S^T/exp of the first OVERLAP chunk-pairs);
            # their PV matmuls are deferred past proj so the in-order PE
            # never blocks on the epilogue's DVE/GPSIMD chain.
            # All four Q' tiles are emitted upfront: they depend only on
            # w2T + x half 0, so they soak up the PE during the bulk
            # (x half 1 / x^T) DMA window at the head. Tile 0's PV then
            # trails its S^T by 4 pairs so the first x^T chunks have time
            # to land without blocking the in-order PE queue.
            for q in range(NQT):
                emit_A(q)
            TRAIL = 4
            for J in range(NPR):
                emit_B_st(0, J)
                if J >= TRAIL:
                    emit_B_pv(0, J - TRAIL)
            for J in range(NPR - TRAIL, NPR):
                emit_B_pv(0, J)
            emit_den_fold(0)
            for q in range(NQT):
                if q + 1 < NQT:
                    lastq = q + 1 == NQT - 1
                    nst = OVERLAP + 2 if lastq else OVERLAP
                    for J in range(nst):
                        emit_B_st(q + 1, J)
                    emit_C_head(q)
                    if lastq:
                        # Last tile: pairs 2..5 fill the proj gaps, and
                        # pairs 0,1 -- whose exps finished ~25us earlier --
                        # close the den/PV accumulations at the very end,
                        # so the final den stop (and the recip->broadcast->
                        # normalize chain behind it) never waits on a
                        # late exp and rides ~3us of exp-independent PE
                        # work instead.
                        emit_C_tail(q, filler=lambda o: emit_B_pv(
                            q + 1, o + 2, start=(o == 0), stop=False))
                        for J in range(OVERLAP + 2, NPR - 1):
                            emit_B(q + 1, J, start=False, stop=False)
                        # final pair: all three den closers go BEFORE the
                        # remaining PV groups so the den stop lands as
                        # early as possible after the last exp.
                        emit_B_st(q + 1, NPR - 1)
                        emit_B_den(q + 1, NPR - 1, start=False, stop=False)
                        emit_B_den(q + 1, 0, start=False, stop=False)
                        emit_B_den(q + 1, 1, start=False, stop=True)
                        emit_B_pv(q + 1, NPR - 1, den=False, start=False,
                                  stop=False)
                        emit_B_pv(q + 1, 0, den=False, start=False,
                                  stop=False)
                        emit_B_pv(q + 1, 1, den=False, start=False,
                                  stop=True)
                    else:
                        emit_C_tail(q, filler=lambda o: emit_B_pv(q + 1, o))
                        for J in range(OVERLAP, NPR):
                            emit_B(q + 1, J)
                        if q + 1 < NQT - 1:
                            emit_den_fold(q + 1)
                else:
                    emit_C_head(q)
                    emit_C_tail(q)

    nc.compile()
    return nc


def _get_compiled():
    global _COMPILED
    if _COMPILED is None:
        _COMPILED = _build()
    return _COMPILED


def kernel(x, qkv_w, qkv_b, proj_w, proj_b):
    global LAST_RESULTS
    import ml_dtypes
    from concourse.bass_utils import run_bass_kernel_spmd

    f8 = ml_dtypes.float8_e4m3fn
    x = np.asarray(x, dtype=np.float32)
    qkv_w = np.asarray(qkv_w, dtype=np.float32)
    qkv_b = np.asarray(qkv_b, dtype=np.float32)
    proj_w = np.asarray(proj_w, dtype=np.float32)
    proj_b = np.asarray(proj_b, dtype=np.float32)

    wq, wk, wv = qkv_w[:C], qkv_w[C:2 * C], qkv_w[2 * C:]
    bq, bv = qkv_b[:C], qkv_b[2 * C:]

    # Host-folded operands (see module docstring). W3 = proj_w @ wv folds
    # the V projection into the output projection: out = W3 @ (X @ attn).
    w2T = np.ascontiguousarray((wq.T @ wk * WSCALE).astype(f8))
    b2 = np.ascontiguousarray(wk.T @ bq * WSCALE)
    w3T = np.ascontiguousarray(((proj_w @ wv).T * WSCALE).astype(f8))
    beff = proj_b + proj_w @ bv

    nc = _get_compiled()

    in_maps = []
    for core in range(NCORES):
        b, h = core // 2, core % 2
        xf = x[b].reshape(C, N)
        xrb = np.ascontiguousarray(
            xf[:, h * QH:(h + 1) * QH] + beff[:, None])
        if h == 0:
            xpf = xf
        else:
            xpf = np.concatenate([xf[:, QH:], xf[:, :QH]], axis=1)
        in_maps.append({
            "xin": np.ascontiguousarray(xpf.astype(f8)),
            "xT": np.ascontiguousarray((xpf.T * 32.0).astype(f8)),
            "xresb": xrb,
            "w2T": w2T, "w3T": w3T, "b2": b2,
        })

    trace = bool(os.environ.get("BASS_KERNEL_TRACE"))
    try:
        res = run_bass_kernel_spmd(
            nc, in_maps, core_ids=list(range(NCORES)), trace=trace)
    except Exception:
        # transient device wedge (e.g. NRT_EXEC_UNIT_UNRECOVERABLE) --
        # one clean retry resolves it in practice
        res = run_bass_kernel_spmd(
            nc, in_maps, core_ids=list(range(NCORES)), trace=False)
    LAST_RESULTS = res

    out = np.empty((B, C, N), dtype=np.float32)
    for core in range(NCORES):
        b, h = core // 2, core % 2
        out[b, :, h * QH:(h + 1) * QH] = res.results[core]["y"]
    return out.reshape(B, C, 64, 64)



# revision 68
# speedup vs baseline: 1.1862x; 1.1862x over previous
"""AttnBlock (VAE-style single-head spatial attention) on 8 Trainium2 cores.

Problem: x[B=4, C=512, H=64, W=64]; qkv 1x1-conv -> attention over N=H*W=4096
tokens -> proj 1x1-conv -> residual add.

Sharding: 8 cores = 4 batch images x 2 query-halves. Each core handles the
full 4096-token context (K/V) of one image and 2048 of its queries. Per-core
x columns are rotated so the query half is always columns [0, 2048) -- the
kj context order is irrelevant (summed over), so the SPMD program is
identical on every core.

Host-side folding (all cheap 512x512 ops):
 - K-bias adds a per-query constant to every logit -> cancels in softmax.
 - V-bias contributes exactly bv to every output column (softmax rows sum to
   1) -> folded with proj_b into the residual tensor xresb = x_q + beff.
 - S^T[kj,qi] = x^T (Wk^T (Wq x_q + bq)) = x^T (W2 x_q + b2) with
   W2 = Wk^T Wq, b2 = Wk^T bq. Scores are computed TRANSPOSED directly from
   x -- no K tensor and no on-chip transposes.
 - Logits are tiny here (|s| < ~1.5), so softmax needs no max-subtraction.

 - The V projection is folded INTO the output projection: out =
   proj(Wv X a) = W3 (X a) with W3 = proj_w @ wv. The kernel therefore
   never materializes V at all -- it accumulates G = X @ p directly from a
   host-provided x^T copy, eliminating all V^T matmuls.

Precision plan: all heavy matmuls run in fp8(e4m3) with DoubleRow perf
mode -- the PE contracts 256 channels per instruction at the same
instruction cost as a 128-deep bf16 matmul, i.e. 2x throughput. w2T/w3T
are scaled by 64 and x^T by 32 on the host so everything lands in e4m3's
normal range; the scales are removed via the exp() scale argument and a
fused 1/2048 multiply in the output op. PSUM accumulation stays fp32 and
the residual add uses an exact fp32 x + beff tensor, so the output error
stays ~2e-4 relative (residual-dominated output).

Per query tile (512 queries), context loop of 16 chunk-PAIRS (2x128
tokens): S^T (2 DoubleRow matmuls/chunk, fp32 PSUM) -> exp on ACT (fp8
out, written into pair-tiles) -> G accumulate (4 DoubleRow matmuls/pair,
lhsT = x^T chunks) + a ones-row DoubleRow matmul accumulating the softmax
denominator in PSUM. In the epilogue the softmax 1/denominator (fast
~51-ULP reciprocal + GPSIMD partition broadcast) is folded into the
G->fp8 normalize mul BEFORE the W3 matmul, so the output-projection PSUM
is exactly 2048x the true attention output and the output stage is a
single fused (pr/2048 + xres) DVE op per co-tile followed by its store.

Pipelining: all four Q' tiles are computed upfront (they only need w2T +
x half 0, covering the bulk-DMA window at the head behind a short PE
warmup spin that also lifts the HAM clock throttle); tile 0's G matmuls
trail its S^T by 4 pairs so the first x^T chunks can land; each tile's
epilogue is covered by the next tile's S^T/exp work with its G matmuls
deferred past the W3 groups; and the LAST tile processes context pairs
0,1 at the very end (their exps finished ~25us earlier) so the final den
stop -> recip -> broadcast -> normalize chain rides exp-independent PE
work. Head input DMA: small critical prefix split across both HWDGE
queues, then junk loads burn the ~10 rotating flow-control semaphores so
the bulk transfers cannot steal DMA engines from the prefix; bulk issue
stays off the scalar queue so Q' bias ACTIVATEs are never stuck behind
0.6-8us DMA descriptor-generation instructions.
"""

import os

import numpy as np

B, C = 4, 512
N = 4096          # H*W tokens
QH = N // 2       # queries per core
QT = 512          # query tile (free dim of most matmuls)
NQT = QH // QT    # 4 query tiles per core
NKC = N // 128    # 32 context chunks
NPR = NKC // 2    # 16 context chunk-pairs
NCC = C // 128    # 4 channel chunks
NCORES = 8
OVERLAP = 4       # next-tile chunk-pairs deferred into the epilogue window
                  # (must equal NCC: one deferred PV group per proj group)
WSCALE = 64.0     # host weight scale into fp8 range

_COMPILED = None
LAST_RESULTS = None  # stashed BassKernelResults for test harness inspection


def _build():
    import concourse.bass as bass  # noqa: F401
    import concourse.mybir as mybir
    import concourse.tile as tile
    from concourse import bacc

    f32 = mybir.dt.float32
    f16 = mybir.dt.float16
    fp8 = mybir.dt.float8e4
    bf16 = mybir.dt.bfloat16
    EXP = mybir.ActivationFunctionType.Exp
    MUL = mybir.AluOpType.mult
    ADD = mybir.AluOpType.add
    DR = mybir.MatmulPerfMode.DoubleRow
    escale = float(C) ** -0.5 / WSCALE
    # G = X @ p is normalized BEFORE the folded output matmul: gmat =
    # G * (1/den) lands at ~0.6-sigma in fp8 (xT carries a host-side x32),
    # and W3 = proj_w @ wv is folded on the host (x64 into fp8), so the
    # PSUM output is exactly 2048x the true attention output. The residual
    # stage folds the 1/2048 into a single fused (pr*s + xres) DVE op.
    OUTSCALE = 1.0 / (WSCALE * 32.0)

    nc = bacc.Bacc("TRN2", target_bir_lowering=False, debug=False,
                   num_devices=NCORES)

    # DRAM I/O (per-core shapes)
    xin = nc.dram_tensor("xin", [C, N], fp8, kind="ExternalInput")
    xT = nc.dram_tensor("xT", [N, C], fp8, kind="ExternalInput")
    xresb = nc.dram_tensor("xresb", [C, QH], f32, kind="ExternalInput")
    w2T = nc.dram_tensor("w2T", [C, C], fp8, kind="ExternalInput")
    w3T = nc.dram_tensor("w3T", [C, C], fp8, kind="ExternalInput")
    b2 = nc.dram_tensor("b2", [C], f32, kind="ExternalInput")
    y = nc.dram_tensor("y", [C, QH], f32, kind="ExternalOutput")

    xr = xin.ap().rearrange("(t p) n -> p t n", p=128)      # [128, 4, 4096]
    xqr = xresb.ap().rearrange("(t p) n -> p t n", p=128)   # [128, 4, 2048]
    yr = y.ap().rearrange("(t p) n -> p t n", p=128)        # [128, 4, 2048]

    with tile.TileContext(nc) as tc:
        with (
            tc.tile_pool(name="singles", bufs=1) as singles,
            tc.tile_pool(name="qp", bufs=4) as qp_pool,
            tc.tile_pool(name="pt", bufs=6) as pt_pool,
            tc.tile_pool(name="hms", bufs=2) as hms_pool,
            tc.tile_pool(name="xres", bufs=2) as xres_pool,
            tc.tile_pool(name="outp", bufs=2) as out_pool,
            tc.tile_pool(name="rc", bufs=2) as rc_pool,
            tc.tile_pool(name="dacc", bufs=2) as dacc_pool,
            tc.tile_pool(name="work", bufs=3, space="PSUM") as work_pool,
            tc.tile_pool(name="hm", bufs=1, space="PSUM") as hm_pool,
            tc.tile_pool(name="den", bufs=1, space="PSUM") as den_pool,
        ):
            # --- PE warmup: ~4.5us of dependency-free matmuls ----------
            # The HAM clock gate needs ~3.4us of sustained PE activity to
            # lift the 1.2 GHz cold throttle; these run during the input
            # DMA wait so the real matmuls start at 2.4 GHz.
            wu_sb = singles.tile([128, QT], bf16)
            nc.vector.memset(wu_sb, 0.0)
            ones_bf = singles.tile([128, 1], bf16)
            nc.vector.memset(ones_bf, 1.0)
            ones16 = singles.tile([128, 1], f16)
            nc.vector.memset(ones16, 1.0)
            wu_keep = singles.tile([1, QT], f32)
            NWU = 13  # bridges engine boot + first-input DMA wait (the
            # head loads land ~11.5us in with dual-queue issue) while
            # giving the PE the ~3us of sustained work its p-state ramp
            # needs
            for w in range(NWU):
                wu_ps = work_pool.tile([1, QT], f32, tag="work", name="wu_ps")
                nc.tensor.matmul(wu_ps, lhsT=ones_bf, rhs=wu_sb)
                if w == NWU - 1:  # keep the chain live against DCE
                    nc.vector.tensor_copy(wu_keep, wu_ps)

            # --- DMAs in consumption-priority order ---------------------
            # The input fetch is descriptor-rate-bound (~3.6ns per 512B x
            # row => ~142GB/s on one queue), so the head loads alternate
            # between BOTH HWDGE queues (sync + scalar; ACT is idle until
            # the first exps) for ~2x aggregate fetch rate.
            _dmaq = [nc.sync, nc.scalar]
            _dqi = [0]

            def head_dma(out, in_):
                _dmaq[_dqi[0] % 2].dma_start(out=out, in_=in_)
                _dqi[0] += 1

            # two pair-tiles: Q'(0)'s first matmul starts after a 128KB DMA
            # instead of the full 256KB weight load
            w2Tr = w2T.ap().rearrange("(t p) m -> p t m", p=128)
            w2T_sb = []
            for tp in range(2):
                ww = singles.tile([128, 2, C], fp8, name=f"w2T{tp}")
                head_dma(out=ww, in_=w2Tr[:, 2 * tp:2 * tp + 2, :])
                w2T_sb.append(ww)
            x0g = [[], [], []]
            for g in range(3):
                for tp in range(2):
                    xx = singles.tile([128, 2, QT], fp8, name=f"x0{g}{tp}")
                    head_dma(out=xx, in_=xr[:, 2 * tp:2 * tp + 2,
                                            g * QT:(g + 1) * QT])
                    x0g[g].append(xx)
            b2_sb = singles.tile([128, NCC], f32)
            nc.sync.dma_start(out=b2_sb,
                              in_=b2.ap().rearrange("(t p) -> p t", p=128))

            # x (fp8): [half][512-col group g] -> [128, 2(ci-pair), 512]
            # per tp. Group 0 of half 0 is its own small pair of tiles so
            # Q'(0) starts after 2x128KB; the rest load as two big tiles
            # per half (1536/2048-col) -- DMA issue instructions cost
            # ~0.6us each regardless of size, so fewer+bigger transfers
            # drain the head ~3x faster.
            xg = [[None] * 4 for _ in range(2)]
            for g in range(3):
                xg[0][g] = x0g[g]
            # Everything below the prefix issues on SYNC ONLY: a DMA
            # issue instruction costs 0.6-8us of engine time (it is
            # descriptor-row-bound), and issues on the scalar queue would
            # delay the Q' bias-add ACTIVATEs behind them, stalling the
            # PE at the head.
            # The ~10 rotating DMA flow-control semaphores make every DMA
            # instruction wait for the completion of the one issued ~10
            # earlier. These tiny junk loads burn the remaining slots so
            # the BULK loads below cannot start transferring (and
            # stealing shared DMA engines) until the startup-critical
            # prefix above has fully landed.
            junk_sb = singles.tile([128, 7], f32, name="junk")
            for jx in range(7):
                c = jx % 4
                nc.sync.dma_start(out=junk_sb[:, jx:jx + 1],
                                  in_=b2.ap().rearrange("(t p) -> p t",
                                                        p=128)[:, c:c + 1])
            # Bulk loads, ordered by first consumption: x half-0 groups 2-3
            # (S^T pair 4 at ~15.5us), x^T chunks 0-7 (tile-0 PV pairs 0-3
            # trail by 4 pairs; x32 host scale), x half-1 (S^T pair 8),
            # then the remaining x^T chunks (PV pair 4 onward).
            xTr = xT.ap().rearrange("(j p) c -> p j c", p=128)
            xT_sb = singles.tile([128, NKC, C], fp8, name="xT")
            xbig = [[None, None], [None, None]]  # [h][tp]
            nc.sync.dma_start(out=xT_sb[:, 0:4, :], in_=xTr[:, 0:4, :])
            for tp in range(2):
                xb = singles.tile([128, 2, QT], fp8, name=f"xb0{tp}")
                nc.sync.dma_start(
                    out=xb, in_=xr[:, 2 * tp:2 * tp + 2, 3 * QT:4 * QT])
                xbig[0][tp] = xb
            nc.sync.dma_start(out=xT_sb[:, 4:8, :], in_=xTr[:, 4:8, :])
            for tp in range(2):
                xb = singles.tile([128, 2, 4 * QT], fp8, name=f"xb1{tp}")
                nc.sync.dma_start(
                    out=xb, in_=xr[:, 2 * tp:2 * tp + 2, QH:QH + 4 * QT])
                xbig[1][tp] = xb
            nc.sync.dma_start(out=xT_sb[:, 8:16, :], in_=xTr[:, 8:16, :])
            nc.sync.dma_start(out=xT_sb[:, 16:24, :], in_=xTr[:, 16:24, :])
            nc.sync.dma_start(out=xT_sb[:, 24:NKC, :],
                              in_=xTr[:, 24:NKC, :])

            def xga(h, g, tp, a=0, b=QT):
                # AP for columns [a,b) of 512-col group g of half h
                if h == 0 and g < 3:
                    return xg[0][g][tp][:, :, a:b]
                off = 0 if h == 0 else g * QT
                return xbig[h][tp][:, :, off + a:off + b]
            w3T_sb = singles.tile([128, NCC, C], fp8)
            nc.sync.dma_start(
                out=w3T_sb,
                in_=w3T.ap().rearrange("(t p) m -> p t m", p=128))
            # fp8 ones for the DoubleRow denominator matmul; width 16 keeps
            # the pair-dim step a multiple of 16 as the DR AP rules require
            ones8 = singles.tile([128, 2, 16], fp8)
            nc.vector.memset(ones8, 1.0)

            def xchunk(j):  # lhsT [ci, 2, kj-cols] fp8 for context chunk j
                h, r = divmod(j, 16)
                g, o = divmod(r, 4)
                return (lambda tp: xga(h, g, tp,
                                       o * 128, (o + 1) * 128))

            S = {}  # per-q live tiles

            def emit_A(q):  # Q' = W2 @ x_q + b2 (scaled by 64, fp8 out)
                # two pair-tiles (not one) so the first S^T matmul of this
                # tile waits only on its own pair's bias adds
                qp_sb = [
                    qp_pool.tile([128, 2, QT], fp8, tag=f"qp{h}",
                                 name=f"qp{q}_{h}")
                    for h in range(2)
                ]
                for m in range(NCC):
                    qp_ps = work_pool.tile([128, QT], f32, tag="work",
                                           name="qp_ps")
                    for tp in range(2):
                        nc.tensor.matmul(
                            qp_ps,
                            lhsT=w2T_sb[tp][:, :, m * 128:(m + 1) * 128],
                            rhs=xga(0, q, tp),
                            start=(tp == 0), stop=(tp == 1),
                            perf_mode=DR,
                        )
                    # bias-add on ACT (Identity), which is idle exactly at
                    # tile boundaries -- the DVE queue there is busy with
                    # the epilogue and would delay the first S^T matmuls
                    nc.scalar.add(
                        qp_sb[m // 2][:, m % 2, :], qp_ps,
                        b2_sb[:, m:m + 1])
                S[q] = {"qp": qp_sb}

            def emit_B_st(q, J):  # S^T + exp of one context chunk-pair
                if J == 0:
                    # four 1-bank tiles (not one 4-bank tile) so each
                    # normalize copy starts as soon as its own bank's last
                    # PV matmul lands, not when the whole group finishes
                    S[q]["hm"] = [
                        hm_pool.tile([128, QT], f32, tag=f"hm{m}",
                                     name=f"hm{q}_{m}")
                        for m in range(NCC)
                    ]
                    S[q]["den"] = den_pool.tile([1, QT], f32, tag="den",
                                                name=f"den{q}")
                    if q < NQT - 1:
                        S[q]["dacc"] = dacc_pool.tile(
                            [128, QT], f16, tag="dacc", name=f"dacc{q}")
                    S[q]["pt"] = {}
                if J == 4:  # prefetch fp32 residual (+bias) slice mid-loop
                    xres_sb = xres_pool.tile([128, NCC, QT], f32, tag="xres",
                                             name=f"xres{q}")
                    nc.sync.dma_start(
                        out=xres_sb, in_=xqr[:, :, q * QT:(q + 1) * QT])
                    S[q]["xres"] = xres_sb
                qp_sb = S[q]["qp"]
                ptd = pt_pool.tile([128, 2, QT], fp8, tag="pt", name="ptd")
                for jj in range(2):
                    j = 2 * J + jj
                    xs = xchunk(j)
                    st_ps = work_pool.tile([128, QT], f32, tag="work",
                                           name="st_ps")
                    for tp in range(2):
                        nc.tensor.matmul(
                            st_ps, lhsT=xs(tp),
                            rhs=qp_sb[tp],
                            start=(tp == 0), stop=(tp == 1),
                            perf_mode=DR,
                        )
                    nc.scalar.activation(ptd[:, jj, :], st_ps, EXP,
                                         scale=escale)
                S[q]["pt"][J] = ptd

            def emit_B_den(q, J, start=None, stop=None):
                # sum-exp accumulate for one chunk-pair. On tiles
                # 0..NQT-2 the EVEN pairs accumulate as DVE adds into an
                # fp16 SBUF tile (the DVE has ~4x slack vs the PE; fp16
                # keeps its 2x 16-bit path and den~4500 only needs ~0.3%
                # accuracy) and only the ODD pairs stay as PE ones-matmuls
                # -- halving the PE's den cost. A single cheap fp16
                # colsum matmul (emit_den_fold) folds the DVE accumulator
                # into the same den PSUM group at tile end. The last tile
                # keeps the all-PE path: its den feeds the latency-
                # critical final recip->broadcast->normalize chain.
                if q < NQT - 1 and J % 2 == 0:
                    ptd = S[q]["pt"][J]
                    acc = S[q]["dacc"]
                    if J == 0:
                        nc.vector.tensor_add(acc, ptd[:, 0, :],
                                             ptd[:, 1, :])
                    else:
                        nc.vector.tensor_add(acc, acc, ptd[:, 0, :])
                        nc.vector.tensor_add(acc, acc, ptd[:, 1, :])
                    return
                if q < NQT - 1:
                    start = J == 1
                    stop = False
                nc.tensor.matmul(
                    S[q]["den"],
                    lhsT=ones8[:, :, 0:1],
                    rhs=S[q]["pt"][J],
                    start=(J == 0) if start is None else start,
                    stop=(J == NPR - 1) if stop is None else stop,
                    perf_mode=DR,
                    skip_group_check=True,
                )

            def emit_den_fold(q):
                # close tile q's den PSUM group: one plain fp16 matmul
                # adds the DVE accumulator's column sums. Emitted AFTER
                # the next tile's S^T batch so the in-order PE has cover
                # while the last DVE adds drain.
                nc.tensor.matmul(S[q]["den"], lhsT=ones16,
                                 rhs=S[q]["dacc"], start=False, stop=True,
                                 skip_group_check=True)

            def emit_B_pv(q, J, den=True, start=None, stop=None):
                # PV accumulate (+ sum-exp). start/stop override the
                # J-based accumulation flags when the last tile's pairs
                # are processed out of order.
                if den:
                    emit_B_den(q, J, start=start, stop=stop)
                hm_ps = S[q]["hm"]
                ptd = S[q]["pt"].pop(J)
                for m in range(NCC):
                    nc.tensor.matmul(
                        hm_ps[m],
                        lhsT=xT_sb[:, 2 * J:2 * J + 2,
                                   m * 128:(m + 1) * 128],
                        rhs=ptd,
                        start=(J == 0) if start is None else start,
                        stop=(J == NPR - 1) if stop is None else stop,
                        perf_mode=DR,
                        skip_group_check=True,
                    )

            def emit_B(q, J, **fl):
                emit_B_st(q, J)
                emit_B_pv(q, J, **fl)

            def emit_C_head(q):
                den_ps = S[q]["den"]
                rec_sb = rc_pool.tile([1, QT], f32, tag="rec",
                                      name=f"rec{q}")
                # ~51-ULP approx (rel err ~4e-6) at 5x the Newton recip
                # speed; den ~ 4096 is far from every undefined edge case.
                # (GPSIMD cannot read PSUM, so recip must precede broadcast.)
                nc.vector.reciprocal_approx_fast(out=rec_sb, in_=den_ps)
                rbc_sb = rc_pool.tile([128, QT], f32, tag="rbc",
                                      name=f"rbc{q}")
                nc.gpsimd.partition_broadcast(rbc_sb, rec_sb)
                # two pair-tiles so each proj DoubleRow matmul waits only on
                # its own pair's normalize muls, not all four
                hmat_sb = [
                    hms_pool.tile([128, 2, QT], fp8, tag=f"hms{h}",
                                  name=f"hms{q}_{h}")
                    for h in range(2)
                ]
                for m in range(NCC):
                    dst = hmat_sb[m // 2][:, m % 2, :]
                    nc.vector.tensor_mul(dst, S[q]["hm"][m], rbc_sb)
                S[q]["hmat"] = hmat_sb

            def emit_C_tail(q, filler=None):
                # proj, then ONE fused (pr*1/4096 + xres) DVE op per
                # co-tile, then store. `filler(o)` injects independent PE
                # work (the deferred next-tile PV groups) between proj
                # groups so the in-order PE never waits on the DVE draining
                # a shared PSUM work slot.
                hmat_sb, xres_sb = S[q]["hmat"], S[q]["xres"]
                out_sb = out_pool.tile([128, NCC, QT], f32, tag="out",
                                       name=f"out{q}")
                for o in range(NCC):
                    pr_ps = work_pool.tile([128, QT], f32, tag="work",
                                           name="pr_ps")
                    for tp in range(2):
                        nc.tensor.matmul(
                            pr_ps,
                            lhsT=w3T_sb[:, 2 * tp:2 * tp + 2,
                                        o * 128:(o + 1) * 128],
                            rhs=hmat_sb[tp],
                            start=(tp == 0), stop=(tp == 1),
                            perf_mode=DR,
                        )
                    # last tile: the very last co-tile drains in two halves
                    # so the final residual op + DMA transfer are half-size
                    H = QT // 2
                    parts = [(0, QT)] if (filler is not None
                                          or o < NCC - 1) else [(0, H),
                                                                (H, QT)]
                    for a, bnd in parts:
                        nc.vector.scalar_tensor_tensor(
                            out=out_sb[:, o, a:bnd], in0=pr_ps[:, a:bnd],
                            scalar=OUTSCALE, in1=xres_sb[:, o, a:bnd],
                            op0=MUL, op1=ADD)
                        # per-co-tile store so output streams out during
                        # the remaining epilogue instead of after all of it.
                        # Last tile alternates the two HWDGE queues (ACT is
                        # idle by then) so the final flush is half as deep.
                        eng = (nc.scalar if filler is None and o % 2
                               else nc.sync)
                        eng.dma_start(
                            out=yr[:, o, q * QT + a:q * QT + bnd],
                            in_=out_sb[:, o, a:bnd])
                    if filler is not None:
                        filler(o)
                del S[q]

            # Pipeline: during tile q's epilogue (denominator -> normalize
            # -> proj), the PE stream holds only dependency-free work from
            # tile q+1 (Q' and S^T/exp of the first OVERLAP chunk-pairs);
            # their PV matmuls are deferred past proj so the in-order PE
            # never blocks on the epilogue's DVE/GPSIMD chain.
            # All four Q' tiles are emitted upfront: they depend only on
            # w2T + x half 0, so they soak up the PE during the bulk
            # (x half 1 / x^T) DMA window at the head. Tile 0's PV then
            # trails its S^T by 4 pairs so the first x^T chunks have time
            # to land without blocking the in-order PE queue.
            for q in range(NQT):
                emit_A(q)
            TRAIL = 4
            for J in range(NPR):
                emit_B_st(0, J)
                if J >= TRAIL:
                    emit_B_pv(0, J - TRAIL)
            for J in range(NPR - TRAIL, NPR):
                emit_B_pv(0, J)
            emit_den_fold(0)
            for q in range(NQT):
                if q + 1 < NQT:
                    lastq = q + 1 == NQT - 1
                    nst = OVERLAP + 2 if lastq else OVERLAP
                    for J in range(nst):
                        emit_B_st(q + 1, J)
                    emit_C_head(q)
                    if lastq:
                        # Last tile: pairs 2..5 fill the proj gaps, and
                        # pairs 0,1 -- whose exps finished ~25us earlier --
                        # close the den/PV accumulations at the very end,
                        # so the final den stop (and the recip->broadcast->
                        # normalize chain behind it) never waits on a
                        # late exp and rides ~3us of exp-independent PE
                        # work instead.
                        emit_C_tail(q, filler=lambda o: emit_B_pv(
                            q + 1, o + 2, start=(o == 0), stop=False))
                        for J in range(OVERLAP + 2, NPR - 1):
                            emit_B(q + 1, J, start=False, stop=False)
                        # final pair: all three den closers go BEFORE the
                        # remaining PV groups so the den stop lands as
                        # early as possible after the last exp.
                        emit_B_st(q + 1, NPR - 1)
                        emit_B_den(q + 1, NPR - 1, start=False, stop=False)
                        emit_B_den(q + 1, 0, start=False, stop=False)
                        emit_B_den(q + 1, 1, start=False, stop=True)
                        emit_B_pv(q + 1, NPR - 1, den=False, start=False,
                                  stop=False)
                        emit_B_pv(q + 1, 0, den=False, start=False,
                                  stop=False)
                        emit_B_pv(q + 1, 1, den=False, start=False,
                                  stop=True)
                    else:
                        emit_C_tail(q, filler=lambda o: emit_B_pv(q + 1, o))
                        for J in range(OVERLAP, NPR):
                            emit_B(q + 1, J)
                        if q + 1 < NQT - 1:
                            emit_den_fold(q + 1)
                else:
                    emit_C_head(q)
                    emit_C_tail(q)

    nc.compile()
    return nc


def _get_compiled():
    global _COMPILED
    if _COMPILED is None:
        _COMPILED = _build()
    return _COMPILED


def kernel(x, qkv_w, qkv_b, proj_w, proj_b):
    global LAST_RESULTS
    import ml_dtypes
    from concourse.bass_utils import run_bass_kernel_spmd

    f8 = ml_dtypes.float8_e4m3fn
    x = np.asarray(x, dtype=np.float32)
    qkv_w = np.asarray(qkv_w, dtype=np.float32)
    qkv_b = np.asarray(qkv_b, dtype=np.float32)
    proj_w = np.asarray(proj_w, dtype=np.float32)
    proj_b = np.asarray(proj_b, dtype=np.float32)

    wq, wk, wv = qkv_w[:C], qkv_w[C:2 * C], qkv_w[2 * C:]
    bq, bv = qkv_b[:C], qkv_b[2 * C:]

    # Host-folded operands (see module docstring). W3 = proj_w @ wv folds
    # the V projection into the output projection: out = W3 @ (X @ attn).
    w2T = np.ascontiguousarray((wq.T @ wk * WSCALE).astype(f8))
    b2 = np.ascontiguousarray(wk.T @ bq * WSCALE)
    w3T = np.ascontiguousarray(((proj_w @ wv).T * WSCALE).astype(f8))
    beff = proj_b + proj_w @ bv

    nc = _get_compiled()

    in_maps = []
    for core in range(NCORES):
        b, h = core // 2, core % 2
        xf = x[b].reshape(C, N)
        xrb = np.ascontiguousarray(
            xf[:, h * QH:(h + 1) * QH] + beff[:, None])
        if h == 0:
            xpf = xf
        else:
            xpf = np.concatenate([xf[:, QH:], xf[:, :QH]], axis=1)
        in_maps.append({
            "xin": np.ascontiguousarray(xpf.astype(f8)),
            "xT": np.ascontiguousarray((xpf.T * 32.0).astype(f8)),
            "xresb": xrb,
            "w2T": w2T, "w3T": w3T, "b2": b2,
        })

    trace = bool(os.environ.get("BASS_KERNEL_TRACE"))
    try:
        res = run_bass_kernel_spmd(
            nc, in_maps, core_ids=list(range(NCORES)), trace=trace)
    except Exception:
        # transient device wedge (e.g. NRT_EXEC_UNIT_UNRECOVERABLE) --
        # one clean retry resolves it in practice
        res = run_bass_kernel_spmd(
            nc, in_maps, core_ids=list(range(NCORES)), trace=False)
    LAST_RESULTS = res

    out = np.empty((B, C, N), dtype=np.float32)
    for core in range(NCORES):
        b, h = core // 2, core % 2
        out[b, :, h * QH:(h + 1) * QH] = res.results[core]["y"]
    return out.reshape(B, C, 64, 64)

